# revision 1
# baseline (speedup 1.0000x reference)
"""Multi-head attention (B=4, S=2048, D=1024, H=16, DK=64) on 8 TRN2 cores.

Sharding: core c = (b, g) with b = c//2 (data parallel on batch) and g = c%2
(tensor parallel on heads: 8 heads / 512 d' columns per group). Host sums the
two partial output projections per batch and adds bo.

v2 changes vs baseline (all bf16; fp8 rejected on accuracy grounds):
  - Q/K bias folded into the PSUM->SBUF eviction (DVE tensor_scalar add with
    a per-partition bias column) -- kills 32 K=1 bias matmuls.
  - JIT startup: only KT(0,0) + QT(0,0) run before the first scores; all
    other K/Q/V projection chains are deadline fillers inside the unit loop,
    so the ACT exp stream starts at ~10us instead of ~40us.
  - Separate PSUM pools: ps_sc (scores, 2x[128,1024] = 4 banks), ps_at
    (AV accumulators, 2x[65,512]), ps_mm (proj/outproj chains, 2x[128,512])
    -- avoids ring stalls between long-lived AV accumulators and fillers.
  - xv/xq staged through small rings instead of full tensors (SBUF budget).
  - Weight DMAs chunked so the first KT/QT chains start after ~1.5MB.
"""

import os
import sys
import time
import types

sys.path.insert(0, "/opt/trn_rl_repo")

import numpy as np
import ml_dtypes


def _install_axon_hooks():
    import antenv

    if "antenv.axon_hooks" in sys.modules:
        return
    hooks = types.ModuleType("antenv.axon_hooks")
    hooks._hook = None
    hooks.set_axon_ntff_profile_hook = lambda h: setattr(hooks, "_hook", h)
    hooks.get_axon_ntff_profile_hook = lambda: hooks._hook
    sys.modules["antenv.axon_hooks"] = hooks
    antenv.axon_hooks = hooks
    try:
        from trn_agent_boot.trn_boot import _ntff_profile_via_ctypes

        hooks.set_axon_ntff_profile_hook(
            _ntff_profile_via_ctypes("/opt/axon/libaxon_pjrt.so")
        )
    except Exception:
        pass


_install_axon_hooks()

import concourse.bacc as bacc
import concourse.bass as bass
import concourse.tile as tile
from concourse import mybir
from concourse import bass_utils
from concourse.bass_utils import run_bass_kernel_spmd

bass_utils.upload_artifacts = lambda tmpdir: tmpdir

BF16 = mybir.dt.bfloat16
F32 = mybir.dt.float32
ALU = mybir.AluOpType

B, S, D = 4, 2048, 1024
H, DK = 16, 64
N_CORES = 8
HC = H // N_CORES * B  # heads per core = 8
DPC = HC * DK  # d' columns per core = 512

LAST_EXEC_TIME_NS = None


def build_program(s=S, dm=D, hc=HC, e=D):
    dk = DK
    dpc = hc * dk
    pairs = hc // 2
    dt_n = dm // 128  # contraction tiles for projections (8)
    st_n = s // 128  # k-tiles (16)
    qc_n = s // 512  # q-chunks (4)
    ec_n = e // 512  # out-proj column chunks (2)

    nc = bacc.Bacc("TRN2", target_bir_lowering=False, debug=False,
                   num_devices=N_CORES)

    xqT = nc.dram_tensor("xqT", [dm, s], BF16, kind="ExternalInput")
    xkT = nc.dram_tensor("xkT", [dm, s], BF16, kind="ExternalInput")
    xvT = nc.dram_tensor("xvT", [dm, s], BF16, kind="ExternalInput")
    wq = nc.dram_tensor("wq", [dm, dpc], BF16, kind="ExternalInput")
    wk = nc.dram_tensor("wk", [dm, dpc], BF16, kind="ExternalInput")
    wv = nc.dram_tensor("wv", [dm, dpc], BF16, kind="ExternalInput")
    wo = nc.dram_tensor("wo", [dpc, e], BF16, kind="ExternalInput")
    bq = nc.dram_tensor("bq", [dpc], F32, kind="ExternalInput")
    bk = nc.dram_tensor("bk", [dpc], F32, kind="ExternalInput")
    out = nc.dram_tensor("out", [s, e], F32, kind="ExternalOutput")

    def step(qc, pr, kk):
        return (qc * pairs + pr) * st_n + kk

    with tile.TileContext(nc) as tc:
        with (
            tc.tile_pool(name="singles", bufs=1) as singles,
            tc.tile_pool(name="xkin", bufs=1) as xkin,
            tc.tile_pool(name="xqin", bufs=2) as xqin,
            tc.tile_pool(name="xvin", bufs=4) as xvin,
            tc.tile_pool(name="expst", bufs=6) as expst_pool,
            tc.tile_pool(name="atn", bufs=hc) as atn_pool,
            tc.tile_pool(name="small", bufs=2) as small,
            tc.tile_pool(name="outsb", bufs=3) as outsb_pool,
            tc.tile_pool(name="ps_sc", bufs=2, space="PSUM") as ps_sc,
            tc.tile_pool(name="ps_at", bufs=2, space="PSUM") as ps_at,
            tc.tile_pool(name="ps_mm", bufs=2, space="PSUM") as ps_mm,
            tc.tile_pool(name="dramb", bufs=4, space="DRAM") as dramb,
        ):
            # ---- persistent SBUF tensors ----
            qt_sb = singles.tile([128, pairs, s], BF16, tag="qt")
            kt_sb = singles.tile([128, pairs, s], BF16, tag="kt")
            vn_sb = singles.tile([128, st_n, hc, dk + 1], BF16, tag="vn")
            wq_sb = singles.tile([128, dt_n, dpc], BF16, tag="wq")
            wk_sb = singles.tile([128, dt_n, dpc], BF16, tag="wk")
            wv_sb = singles.tile([128, dt_n, dpc], BF16, tag="wv")
            wo_sb = singles.tile([128, pairs, e], BF16, tag="wo")
            bqc_sb = singles.tile([128, pairs], F32, tag="bqc")
            bkc_sb = singles.tile([128, pairs], F32, tag="bkc")
            ones_sb = singles.tile([128, 512], BF16, tag="ones")
            xk_sb = xkin.tile([128, qc_n, dt_n, 512], BF16, tag="xk")

            # ---- DMA issue order: tiny bias vectors first (they must not
            # queue behind MB-sized transfers), then first-exp dependencies.
            nc.sync.dma_start(
                out=bqc_sb, in_=bq.ap().rearrange("(pr p) -> p pr", p=128))
            nc.sync.dma_start(
                out=bkc_sb, in_=bk.ap().rearrange("(pr p) -> p pr", p=128))
            wk_src = wk.ap().rearrange("(t p) n -> p t n", p=128)
            wq_src = wq.ap().rearrange("(t p) n -> p t n", p=128)
            xk_src = xkT.ap().rearrange("(t p) n -> p t n", p=128)
            xq_src = xqT.ap().rearrange("(t p) n -> p t n", p=128)
            xv_src = xvT.ap().rearrange("(t p) n -> p t n", p=128)

            # wk chunk 0 (cols 0:128 -> KT pair 0), xk block 0, wq chunk 0,
            # xq block 0 -> first scores at ~10us.
            nc.sync.dma_start(out=wk_sb[:, :, 0:128], in_=wk_src[:, :, 0:128])
            nc.sync.dma_start(out=xk_sb[:, 0, :, :], in_=xk_src[:, :, 0:512])
            nc.sync.dma_start(out=wq_sb[:, :, 0:128], in_=wq_src[:, :, 0:128])
            xq_blocks = {}
            xq_blocks[0] = xqin.tile([128, dt_n, 512], BF16, tag="xq", name="xqb")
            nc.sync.dma_start(out=xq_blocks[0], in_=xq_src[:, :, 0:512])
            # V path (first AV needs it at ~14us)
            nc.sync.dma_start(
                out=wv_sb, in_=wv.ap().rearrange("(t p) n -> p t n", p=128))
            xv_blocks = {}

            def ensure_xv(j):
                # stage xv in 256-col blocks through a 4-slot ring
                while len(xv_blocks) <= min(j + 2, s // 256 - 1):
                    nj = len(xv_blocks)
                    blk = xvin.tile([128, dt_n, 256], BF16, tag="xv", name="xvb")
                    nc.sync.dma_start(
                        out=blk, in_=xv_src[:, :, nj * 256:(nj + 1) * 256])
                    xv_blocks[nj] = blk

            ensure_xv(0)
            # rest of xk, then remaining weight chunks
            for j in range(1, qc_n):
                nc.sync.dma_start(
                    out=xk_sb[:, j, :, :],
                    in_=xk_src[:, :, j * 512:(j + 1) * 512])
            nc.sync.dma_start(
                out=wk_sb[:, :, 128:dpc], in_=wk_src[:, :, 128:dpc])
            nc.sync.dma_start(
                out=wq_sb[:, :, 128:dpc], in_=wq_src[:, :, 128:dpc])
            nc.sync.dma_start(
                out=wo_sb, in_=wo.ap().rearrange("(a p) e -> p a e", p=128))

            nc.vector.memset(ones_sb, 1.0)
            nc.vector.memset(vn_sb[:, :, :, dk:dk + 1], 1.0)

            # Warm-up exp ACT: overlap the ~1.3us ACT_TABLE_LOAD with the
            # first projection chains.
            warm_sb = singles.tile([128, 32], F32, tag="warm")
            nc.scalar.activation(
                warm_sb, ones_sb[:, 0:32], mybir.ActivationFunctionType.Exp)

            # ---- helper emitters ----
            def qk_chain(w_sb, b_sb, xs, dst, p, qcc):
                """QT/KT chain: 8 matmuls + biased eviction (no bias mm)."""
                ps = ps_mm.tile([128, 512], F32, tag="ps")
                for t in range(dt_n):
                    nc.tensor.matmul(
                        ps,
                        w_sb[:, t, p * 128:(p + 1) * 128],
                        xs[:, t, :],
                        start=(t == 0),
                        stop=(t == dt_n - 1),
                    )
                nc.vector.tensor_scalar(
                    dst[:, p, qcc * 512:(qcc + 1) * 512], ps,
                    b_sb[:, p:p + 1], None, ALU.add)

            def qk_gen(w_sb, b_sb, xs, dst, p, qcc):
                ps = ps_mm.tile([128, 512], F32, tag="ps")
                for t in range(dt_n):
                    nc.tensor.matmul(
                        ps,
                        w_sb[:, t, p * 128:(p + 1) * 128],
                        xs[:, t, :],
                        start=(t == 0),
                        stop=(t == dt_n - 1),
                    )
                    yield
                nc.vector.tensor_scalar(
                    dst[:, p, qcc * 512:(qcc + 1) * 512], ps,
                    b_sb[:, p:p + 1], None, ALU.add)
                yield

            def v_gen(st):
                """V chain for k-tile st: 8 matmuls + ones-row bias + evict."""
                ensure_xv(st // 2)
                blk = xv_blocks[st // 2]
                off = (st % 2) * 128
                ps = ps_mm.tile([128, 512], F32, tag="ps")
                for t in range(dt_n):
                    nc.tensor.matmul(
                        ps,
                        blk[:, t, off:off + 128],
                        wv_sb[:, t, :],
                        start=(t == 0),
                        stop=(t == dt_n - 1),
                    )
                    yield
                nc.vector.tensor_copy(
                    vn_sb[:, st, :, 0:dk],
                    ps.rearrange("p (h d) -> p h d", d=dk),
                )
                yield

            def outproj_gen(atn_q, qcc, qt_i, ecc):
                esl = slice(ecc * 512, (ecc + 1) * 512)
                q0 = qcc * 4 + qt_i
                o_ps = ps_mm.tile([128, 512], F32, tag="ps")
                for p in range(pairs):
                    nc.tensor.matmul(
                        o_ps,
                        atn_q[p][:, qt_i * 128:(qt_i + 1) * 128],
                        wo_sb[:, p, esl],
                        start=(p == 0),
                        stop=(p == pairs - 1),
                    )
                    yield
                o_sb = outsb_pool.tile([128, 512], F32, tag="o")
                nc.vector.tensor_copy(o_sb, o_ps)
                nc.sync.dma_start(
                    out=out.ap()[q0 * 128:(q0 + 1) * 128, esl], in_=o_sb)
                yield

            class FillerQueue:
                def __init__(self):
                    self.tasks = []  # (gen, deadline_step or None)

                def add(self, gen, deadline=None):
                    self.tasks.append((gen, deadline))

                def pump(self, n):
                    while n > 0 and self.tasks:
                        try:
                            next(self.tasks[0][0])
                            n -= 1
                        except StopIteration:
                            self.tasks.pop(0)

                def fence(self, cur):
                    while self.tasks and any(
                        dl is not None and dl <= cur for _, dl in self.tasks
                    ):
                        self.pump(1)

                def drain(self):
                    while self.tasks:
                        self.pump(1000)

            fill = FillerQueue()

            # ---- stage A: just enough for the first scores ----
            qk_chain(wk_sb, bkc_sb, xk_sb[:, 0, :, :], kt_sb, 0, 0)
            qk_chain(wq_sb, bqc_sb, xq_blocks[0], qt_sb, 0, 0)

            # stage-A remainder as deadline fillers (all within qc0):
            # V(st) before AV(st) (consumed at kk=st+1 of pair 0);
            # interleave KT(0, c>=1) (needed at kk=4c) between V chains.
            fill.add(v_gen(0), deadline=step(0, 0, 0))
            fill.add(v_gen(1), deadline=step(0, 0, 1))
            fill.add(v_gen(2), deadline=step(0, 0, 2))
            for c in range(1, qc_n):
                fill.add(qk_gen(wk_sb, bkc_sb, xk_sb[:, c, :, :], kt_sb, 0, c),
                         deadline=step(0, 0, 4 * c - 1))
                fill.add(v_gen(2 * c + 1), deadline=step(0, 0, 2 * c + 1))
                fill.add(v_gen(2 * c + 2), deadline=step(0, 0, 2 * c + 2))
            for st in range(9, st_n):
                fill.add(v_gen(st), deadline=step(0, 0, st))
            # later pairs' KT/QT (kt/qt needed when unit (0, p) starts)
            for p in range(1, pairs):
                fill.add(qk_gen(wq_sb, bqc_sb, xq_blocks[0], qt_sb, p, 0),
                         deadline=step(0, p, 0))
                for c in range(qc_n):
                    fill.add(
                        qk_gen(wk_sb, bkc_sb, xk_sb[:, c, :, :], kt_sb, p, c),
                        deadline=step(0, p, max(4 * c - 1, 0)))

            # ---- stages B+C interleaved over q-chunks ----
            prev_atn = None
            prev_prev_atn = None
            pending_norm = None
            for qc in range(qc_n):
                qsl = slice(qc * 512, (qc + 1) * 512)
                last = qc == qc_n - 1
                atn_q = []
                rs_sb = small.tile([hc, 512], F32, tag="rs")

                # prefetch xq block qc+1 and enqueue its QT chains
                if qc + 1 < qc_n:
                    blkq = xqin.tile([128, dt_n, 512], BF16, tag="xq")
                    nc.sync.dma_start(
                        out=blkq,
                        in_=xq_src[:, :, (qc + 1) * 512:(qc + 2) * 512])
                    xq_blocks[qc + 1] = blkq
                    for pp in range(pairs):
                        fill.add(
                            qk_gen(wq_sb, bqc_sb, blkq, qt_sb, pp, qc + 1),
                            deadline=step(qc + 1, pp, 0))
                if last and pending_norm is not None:
                    # the last q-chunk's outproj fillers must follow the
                    # qc-2 normalization emission
                    pending_norm()
                    pending_norm = None
                op_dls = [(1, 15), (1, 15), (2, 7), (2, 15),
                          (3, 3), (3, 7), (3, 11), (3, 15)]
                if last and prev_atn is not None:
                    seq_total = 4 * ec_n
                    n_defer = 4
                    for sq in range(seq_total - n_defer):
                        dp, dkk = op_dls[sq]
                        fill.add(outproj_gen(
                            prev_atn, qc - 1, sq // ec_n, sq % ec_n),
                            deadline=step(qc, min(dp, pairs - 1), dkk))

                for pr in range(pairs):
                    at_A = ps_at.tile([65, 512], F32, tag="at")
                    at_B = ps_at.tile([65, 512], F32, tag="at")
                    pipe = None  # (kk, exp_sb) awaiting its AT matmuls

                    def emit_at(kk, e_sb):
                        nc.tensor.matmul(
                            at_A,
                            vn_sb[:, kk, 2 * pr, :],
                            e_sb[:, 0:512],
                            start=(kk == 0),
                            stop=(kk == st_n - 1),
                        )
                        nc.tensor.matmul(
                            at_B,
                            vn_sb[:, kk, 2 * pr + 1, :],
                            e_sb[:, 512:1024],
                            start=(kk == 0),
                            stop=(kk == st_n - 1),
                        )

                    for kk in range(st_n):
                        fill.fence(step(qc, pr, kk))
                        sc_ps = ps_sc.tile([128, 1024], F32, tag="sc")
                        ksl = slice(kk * 128, (kk + 1) * 128)
                        nc.tensor.matmul(
                            sc_ps[:, 0:512],
                            kt_sb[0:64, pr, ksl],
                            qt_sb[0:64, pr, qsl],
                            start=True,
                            stop=True,
                        )
                        nc.tensor.matmul(
                            sc_ps[:, 512:1024],
                            kt_sb[64:128, pr, ksl],
                            qt_sb[64:128, pr, qsl],
                            start=True,
                            stop=True,
                        )
                        exp_sb = expst_pool.tile([128, 1024], BF16, tag="e")
                        nc.scalar.activation(
                            exp_sb, sc_ps,
                            mybir.ActivationFunctionType.Exp,
                            scale=1.0 / np.sqrt(dk),
                        )
                        if pipe is not None:
                            emit_at(*pipe)
                        pipe = (kk, exp_sb)
                        if last and pr == 0 and kk < 10:
                            pass  # let the qc-2 norm chain land before its
                                  # out-projections can head-block the FIFO
                        else:
                            fill.pump(2 if last or pr >= pairs - 2 else (1 + (kk & 1)))
                    emit_at(*pipe)

                    # pair tile: head A direct DVE copy to partitions 0:64,
                    # head B shifted to 64:128 via SBUF->SBUF DMA
                    atn_pair = atn_pool.tile([128, 512], BF16, tag="atn")
                    nc.vector.tensor_copy(atn_pair[0:64, :], at_A[0:64, :])
                    btmp = small.tile([64, 512], BF16, tag="btmp")
                    nc.vector.tensor_copy(btmp, at_B[0:64, :])
                    nc.sync.dma_start(out=atn_pair[64:128, :], in_=btmp)
                    atn_q.append(atn_pair)
                    if pr == 0 and pending_norm is not None:
                        pending_norm()
                        pending_norm = None
                        if not last and prev_atn is not None:
                            for sq in range(4 * ec_n):
                                dp, dkk = op_dls[sq]
                                fill.add(outproj_gen(
                                    prev_atn, qc - 1, sq // ec_n, sq % ec_n),
                                    deadline=step(qc, dp, dkk))
                    for h, at_ps, lo in (
                        (2 * pr, at_A, 0),
                        (2 * pr + 1, at_B, 64),
                    ):
                        prng = slice(lo, lo + 64)
                        if not last:
                            rs_row = small.tile([65, 512], F32, tag="rsrow")
                            nc.vector.tensor_copy(
                                rs_row[64:65, :], at_ps[64:65, :])
                            nc.sync.dma_start(
                                out=rs_sb[h:h + 1, :], in_=rs_row[64:65, :])
                        else:
                            # per-head normalization, pipelined under later
                            # units so nothing gates the tail out-projections
                            rec_row = small.tile([65, 512], F32, tag="recr")
                            nc.vector.reciprocal(
                                out=rec_row[64:65, :], in_=at_ps[64:65, :])
                            rd1 = dramb.tile([1, 512], F32, tag="rd1")
                            nc.sync.dma_start(out=rd1, in_=rec_row[64:65, :])
                            bc_sb = small.tile([128, 512], F32, tag="bc")
                            bcast_src = bass.AP(
                                tensor=rd1.tensor,
                                offset=rd1.offset,
                                ap=[[0, 64]] + list(rd1.ap[1:]),
                            )
                            nc.sync.dma_start(
                                out=bc_sb[prng, :], in_=bcast_src)
                            nc.gpsimd.tensor_mul(
                                atn_pair[prng, :],
                                atn_pair[prng, :],
                                bc_sb[prng, :],
                            )

                fill.fence(step(qc, pairs - 1, st_n - 1))

                # batched softmax normalization for the whole q-chunk,
                # deferred into the next q-chunk (after its first pair unit)
                if not last:
                    def make_norm(rs_sb=rs_sb, atn_q=atn_q):
                        def norm():
                            rec_sb = small.tile([hc, 512], F32, tag="rec")
                            nc.vector.reciprocal(out=rec_sb, in_=rs_sb)
                            rec_dram = dramb.tile([hc, 512], F32, tag="recd")
                            nc.sync.dma_start(out=rec_dram, in_=rec_sb)
                            for h in range(hc):
                                p, lo = h // 2, 64 * (h % 2)
                                prng = slice(lo, lo + 64)
                                row = rec_dram[h:h + 1, :]
                                bc_sb = small.tile([128, 512], F32, tag="bc")
                                bcast_src = bass.AP(
                                    tensor=row.tensor,
                                    offset=row.offset,
                                    ap=[[0, 64]] + list(row.ap[1:]),
                                )
                                nc.sync.dma_start(
                                    out=bc_sb[prng, :], in_=bcast_src)
                                nc.gpsimd.tensor_mul(
                                    atn_q[p][prng, :], atn_q[p][prng, :],
                                    bc_sb[prng, :])
                        return norm
                    pending_norm = make_norm()
                prev_prev_atn = prev_atn
                prev_atn = atn_q

            fill.drain()

            # tail: deferred C(qc_n-2) sequences cover the last per-head
            # normalization chains, then C(qc_n-1)
            tail_fill = FillerQueue()
            if qc_n >= 2:
                for sq in range(4 * ec_n - 4, 4 * ec_n):
                    tail_fill.add(outproj_gen(
                        prev_prev_atn, qc_n - 2, sq // ec_n, sq % ec_n))
            for qt_i in range(4):
                for ecc in range(ec_n):
                    tail_fill.add(outproj_gen(prev_atn, qc_n - 1, qt_i, ecc))
            tail_fill.drain()

    nc.compile()
    return nc


_PROGRAM_CACHE = {}


def _get_program(key):
    if key not in _PROGRAM_CACHE:
        _PROGRAM_CACHE[key] = build_program(*key)
    return _PROGRAM_CACHE[key]


def kernel(queries, keys, values, Wq, bq, Wk, bk, Wv, bv, Wo, bo):
    global LAST_EXEC_TIME_NS
    bf16 = ml_dtypes.bfloat16

    nc = _get_program((S, D, HC, D))

    xT = {}
    for name, arr in (("q", queries), ("k", keys), ("v", values)):
        xT[name] = [
            np.ascontiguousarray(np.asarray(arr[b]).T).astype(bf16)
            for b in range(B)
        ]
    Wq, Wk, Wv, Wo = (np.asarray(w) for w in (Wq, Wk, Wv, Wo))
    bqv, bkv, bvv = (np.asarray(v) for v in (bq, bk, bv))

    in_maps = []
    for c in range(N_CORES):
        b, g = c // 2, c % 2
        csl = slice(g * DPC, (g + 1) * DPC)
        in_maps.append(
            {
                "xqT": xT["q"][b],
                "xkT": xT["k"][b],
                "xvT": xT["v"][b],
                "wq": np.ascontiguousarray(Wq[:, csl]).astype(bf16),
                "wk": np.ascontiguousarray(Wk[:, csl]).astype(bf16),
                "wv": np.ascontiguousarray(Wv[:, csl]).astype(bf16),
                "wo": np.ascontiguousarray(Wo[csl, :]).astype(bf16),
                "bq": np.ascontiguousarray(bqv[csl]).astype(np.float32),
                "bk": np.ascontiguousarray(bkv[csl]).astype(np.float32),
            }
        )

    trace = os.environ.get("KERNEL_TRACE", "0") == "1"
    res = run_bass_kernel_spmd(nc, in_maps, list(range(N_CORES)), trace=trace)
    LAST_EXEC_TIME_NS = res.exec_time_ns

    # bv's contribution commutes through softmax-normalized attention:
    # each head's output gains +bv_h, so the final output gains bv @ Wo.
    bo = np.asarray(bo, dtype=np.float32) + bvv.astype(np.float32) @ Wo.astype(np.float32)
    out = np.empty((B, S, D), dtype=np.float32)
    for b in range(B):
        out[b] = res.results[2 * b]["out"] + res.results[2 * b + 1]["out"] + bo
    return out


if __name__ == "__main__":
    t0 = time.time()
    nc = _get_program((S, D, HC, D))
    print(f"build+compile: {time.time() - t0:.1f}s")

